# revision 56
# baseline (speedup 1.0000x reference)
"""Multi-head attention layer on 8 TRN2 NeuronCores.

Problem: B=4, L=S=2048, D=512, H=8 heads of E=64.
out = softmax(scale * (x_q Wq + bq)(x_k Wk + bk)^T) (x_v Wv + bv) Wo + bo

Sharding: core c = 2*b + j handles batch b, head-half j (4 heads).
Each core computes a partial output projection [2048, 512]; the host sums
the two partials per batch and adds the (bv @ Wo + bo) epilogue.
bk is dropped on-chip (softmax is invariant to a per-row constant shift).

Host prep (layout only, no FLOPs): x inputs are transposed to [D, L] and
cast to bf16 so the kernel needs no on-chip transposes.

Host prep is layout-only (transpose/cast/pre-tiling so every DMA
descriptor covers a full 4KB partition row; the per-queue DMA rate is
descriptor-bound).

Per-core kernel (all matmuls bf16, f32 PSUM accumulation):
  qT    = Wq^T xT + bq  [256e, 2048]  (e on partitions, heads packed 2/ptile)
  kT    = Wk^T xT       [256e, 2048]
  v     = (xT)^T Wv     [2048s, 4, 65] with a trailing ones column per head
  loop qc (q chunks of 512) outer, pr (head pair) inner; per s-tile of 128,
  software-pipelined (scores for stage k+1 are emitted before exp of stage
  k; TWO stages ahead around DVE stages and block starts, where the scalar
  engine would otherwise wait out the 2-slot score-PSUM rotation):
    S^T[s,q]   = kT_h^T @ qT_h       (two row-packed matmuls, tile_position)
    P^T        = exp(scale * S^T)    11 of 16 stages on ScalarE; DVE_ST
                 stages instead use a zero-mean Schraudolph exp on DVE (one
                 tensor_scalar int32(A*s + B); the bf16 high half of the
                 int32 IS exp to ~1.8% rms) so the exp stream runs on two
                 engines concurrently.  The exp is the critical path: per
                 core 16.8M exps at 1/lane/cycle.
    O[q,65]   += P_slice^T @ v_aug_h (transposed PV: 8 matmuls of free size
                 65 instead of 2 of 512 — matmul time is the out free dim —
                 and col 64 accumulates Z per PARTITION q, so softmax
                 normalization becomes a per-partition scalar).  PSUM
                 start=True zeroes the accumulator's whole bank: only the
                 first matmul per bank sets it.
  Drain per (qc, pr): DVE reciprocal of the Z column + 2 copies free the
  PSUM accumulators fast; per-partition tensor_scalar ops normalize into
  o_norm [q, h, e]; PE transposes (deferred into the next block's early
  stages so they never gate its scores) flip to oT [he, q], packed 2 heads
  per 128 partitions.  Output projection per q-tile is then just 2 matmuls
  (contraction 128), emitted inside the next chunk's s-loop; the last
  chunk's runs in a per-q-tile pipelined tail with casts on the (then idle)
  scalar engine.
  out  = oT^T @ Wo -> DRAM (bf16 partials; host sums in f32)
"""

import numpy as np

import concourse.bacc as bacc
import concourse.bass as bass
import concourse.mybir as mybir
import concourse.tile as tile
from concourse.bass_utils import run_bass_kernel_spmd
from concourse.masks import make_identity

B, L, S, D, H = 4, 2048, 2048, 512, 8
E = 64          # head dim
HPC = 4         # heads per core
EC = HPC * E    # 256 model cols per core
P = 128
ST = S // P     # 16 s-tiles
DC = D // P     # 4 d-chunks
QC = 4          # q chunks of 512
QW = 512        # q chunk width
SC = 4          # s chunks of 512 (x dma / projection granularity)
SW = 512
FP32 = mybir.dt.float32
BF16 = mybir.dt.bfloat16
AF = mybir.ActivationFunctionType
VW = E + 1      # v columns per head incl. trailing ones column (gives Z)


def _emit(nc, tc):
    # all inputs pre-tiled on host to the exact SBUF layout so every DMA
    # descriptor covers a full partition row (4KB vs 1KB: the per-queue DMA
    # rate is descriptor-bound)
    xqT = nc.dram_tensor("xqT", [SC, P, DC, SW], BF16, kind="ExternalInput")
    xkT = nc.dram_tensor("xkT", [SC, P, DC, SW], BF16, kind="ExternalInput")
    xvT = nc.dram_tensor("xvT", [SC, P, DC, SW], BF16, kind="ExternalInput")
    wq = nc.dram_tensor("wq", [P, DC, EC], BF16, kind="ExternalInput")
    wk = nc.dram_tensor("wk", [P, DC, EC], BF16, kind="ExternalInput")
    wv = nc.dram_tensor("wv", [P, DC, EC], BF16, kind="ExternalInput")
    wo = nc.dram_tensor("wo", [P, 2, D], BF16, kind="ExternalInput")
    bq = nc.dram_tensor("bq", [P, 2], FP32, kind="ExternalInput")
    out = nc.dram_tensor("out", [L, D], BF16, kind="ExternalOutput")

    const = tc.alloc_tile_pool(name="const", bufs=1)
    wpool = tc.alloc_tile_pool(name="weights", bufs=1)
    big = tc.alloc_tile_pool(name="big", bufs=1)
    xpool = tc.alloc_tile_pool(name="xload", bufs=1)
    psb = tc.alloc_tile_pool(name="pexp", bufs=6)
    zp = tc.alloc_tile_pool(name="znorm", bufs=2)
    ocp = tc.alloc_tile_pool(name="oc", bufs=2)
    psum = tc.alloc_tile_pool(name="psum", bufs=1, space="PSUM")

    # One dma_start per load: a single DMA's descriptors already fan out
    # across all 16 DMA engines, so splitting for bandwidth buys nothing —
    # but every issue costs ~0.6us on the SP sequencer, so loads are merged
    # and ordered needed-first.
    bq_sb = const.tile([P, 2], FP32)
    ident = const.tile([P, P], BF16)

    # weights; layout [128 d_local, dc, EC]
    w_sb = {}
    for name, wt in (("wq", wq), ("wk", wk), ("wv", wv)):
        t = wpool.tile([P, DC, EC], BF16, tag=f"w_{name}", name=f"w_{name}")
        w_sb[name] = t

    def load_w(name, wt, eng):
        eng.dma_start(
            out=w_sb[name][:],
            in_=bass.AP(wt, 0, [[DC * EC, P], [1, DC * EC]]),
        )

    # out projection weights packed 2 heads per 128 partitions: row he of
    # wo_sb[:, pt, :] is Wo row pt*128+he (heads 2pt, 2pt+1 stacked)
    wo_sb = wpool.tile([P, 2, D], BF16, tag="w_wo")

    # x chunk tiles: per (name, sc) a [128, DC, 512] tile
    xch = {"xq": [None] * SC, "xk": [None] * SC, "xv": [None] * SC}

    def load_x(name, dram, sc, eng):
        t = xpool.tile([P, DC, SW], BF16, tag=f"x_{name}_{sc}", name=f"x_{name}_{sc}")
        eng.dma_start(
            out=t[:],
            in_=bass.AP(dram, sc * P * DC * SW, [[DC * SW, P], [1, DC * SW]]),
        )
        xch[name][sc] = t

    # Ramp loads: transfers serialize per HWDGE queue (~2.3us per 512KB),
    # and gpsimd's SWDGE queue is ~3x slower — only tensors needed tens of
    # us in (xk3, wo, xq3) go there.  First-exp critical path: bq/wq/wk +
    # xk0 on SP, xq0 on the scalar queue (free once its engine-state load
    # finishes), both split per d-chunk.
    load_x("xk", xkT, 0, nc.scalar)
    # preload the exp activation-table set during the DMA ramp so the first
    # real exp doesn't pay the ~2.7us ACT_TABLE_LOAD
    warm = const.tile([1, 2], FP32)
    nc.vector.memset(warm[:, 0:1], 0.0)
    nc.scalar.activation(warm[:, 1:2], warm[:, 0:1], AF.Exp)
    load_w("wq", wq, nc.sync)
    load_w("wk", wk, nc.sync)
    load_x("xq", xqT, 0, nc.scalar)
    make_identity(nc, ident[:])
    nc.sync.dma_start(out=bq_sb[:], in_=bass.AP(bq, 0, [[2, P], [1, 2]]))
    load_w("wv", wv, nc.sync)
    load_x("xv", xvT, 0, nc.scalar)
    load_x("xk", xkT, 1, nc.sync)
    load_x("xv", xvT, 1, nc.scalar)
    load_x("xk", xkT, 2, nc.sync)
    load_x("xv", xvT, 2, nc.scalar)
    load_x("xk", xkT, 3, nc.gpsimd)
    load_x("xv", xvT, 3, nc.scalar)
    nc.gpsimd.dma_start(
        out=wo_sb[:], in_=bass.AP(wo, 0, [[2 * D, P], [1, 2 * D]])
    )
    load_x("xq", xqT, 1, nc.sync)
    load_x("xq", xqT, 2, nc.sync)
    load_x("xq", xqT, 3, nc.gpsimd)

    # PE p-state warm-up: throwaway identity matmuls from ~9us until the
    # first projection inputs land (~16.5us), so qT/kT/scores run at ramped
    # clock with no idle gap to reset the p-state
    wu = psum.tile([P, P], FP32, tag="pa", bufs=2, name="warmup")
    for _ in range(70):
        nc.tensor.matmul(wu[:], lhsT=ident[:], rhs=ident[:], start=True, stop=True)

    # persistent activations
    qT = big.tile([P, 2, L], BF16, tag="qT")   # [e_local, ptile, q]
    kT = big.tile([P, 2, S], BF16, tag="kT")
    v_sb = big.tile([P, ST, HPC, VW], BF16, tag="v")  # [s_local, s_tile, h, e+1]
    nc.vector.memset(v_sb[:, :, :, E : E + 1], 1.0)
    oT = big.tile([P, 2, L], BF16, tag="oT")  # [he (2 heads x 64e), pr, q]

    # ---------------- projection emitters ----------------
    def qT_proj(qc, pt):
        ps = psum.tile([P, QW], FP32, tag="pa", bufs=2)
        for dc in range(DC):
            nc.tensor.matmul(
                ps[:],
                lhsT=w_sb["wq"][:, dc, pt * P : (pt + 1) * P],
                rhs=xch["xq"][qc][:, dc, :],
                start=(dc == 0),
                stop=(dc == DC - 1),
            )
        nc.vector.tensor_scalar_add(
            out=qT[:, pt, qc * QW : (qc + 1) * QW],
            in0=ps[:],
            scalar1=bq_sb[:, pt : pt + 1],
        )

    def kT_proj(sc, pt, c0=0, cw=SW):
        ps = psum.tile([P, QW], FP32, tag="pa", bufs=2, name=f"kp_{sc}_{pt}_{c0}")
        for dc in range(DC):
            nc.tensor.matmul(
                ps[:, 0:cw],
                lhsT=w_sb["wk"][:, dc, pt * P : (pt + 1) * P],
                rhs=xch["xk"][sc][:, dc, c0 : c0 + cw],
                start=(dc == 0),
                stop=(dc == DC - 1),
            )
        nc.vector.tensor_copy(
            out=kT[:, pt, sc * SW + c0 : sc * SW + c0 + cw], in_=ps[:, 0:cw]
        )

    def v_proj(st):
        ps = psum.tile([P, EC], FP32, tag="pa", bufs=2)
        for dc in range(DC):
            nc.tensor.matmul(
                ps[:],
                lhsT=xch["xv"][st // 4][:, dc, (st % 4) * P : (st % 4 + 1) * P],
                rhs=w_sb["wv"][:, dc, :],
                start=(dc == 0),
                stop=(dc == DC - 1),
            )
        nc.vector.tensor_copy(
            out=v_sb[:, st, :, 0:E],
            in_=ps[:].rearrange("p (h e) -> p h e", h=HPC),
        )

    def out_proj(qc, qt, dma_eng=None, tag="pa", bufs=2, cast_eng=None):
        ops = psum.tile([P, D], FP32, tag=tag, bufs=bufs, name=f"op_{qc}_{qt}")
        q0 = qc * QW + qt * P
        for pt in range(2):
            nc.tensor.matmul(
                ops[:],
                lhsT=oT[:, pt, q0 : q0 + P],
                rhs=wo_sb[:, pt, :],
                start=(pt == 0),
                stop=(pt == 1),
            )
        o_stage = ocp.tile([P, D], BF16, tag="ostage", bufs=3)
        if cast_eng is None:
            nc.vector.tensor_copy(out=o_stage[:], in_=ops[:])
        else:
            cast_eng.copy(out=o_stage[:], in_=ops[:])
        (dma_eng or nc.sync).dma_start(out=out[q0 : q0 + P, :], in_=o_stage[:])

    # ---------------- attention ----------------
    scale = 1.0 / np.sqrt(E)
    # Schraudolph exp on DVE for DVE_ST s-tiles of each block: exp(scale*s)
    # ~= bf16_high16(int32(A*scale*s + B)); C=482784 zero-means the relative
    # error so the approx s-tiles are not systematically overweighted in the
    # softmax (the ~1.8% rms sawtooth lands on len(DVE_ST)/16 of each row).
    SCH_A = float((1 << 23) / np.log(2.0) * scale)
    SCH_B = float(127 * (1 << 23) + (1 << 15) - 482784)
    # no DVE stages in the PE-bound phase-A chunk (qc==0); 5 per block after
    DVE_ST_OF_QC = {0: (), 1: (3, 6, 9, 12, 14), 2: (3, 6, 9, 12, 14),
                    3: (3, 6, 9, 12, 14)}
    s_tiles = {}

    def emit_scores(qc, pr, st):
        s_ps = psum.tile(
            [P, 2 * QW], FP32, tag="ps", bufs=2, name=f"s_{pr}_{qc}_{st}"
        )
        for i in range(2):
            nc.tensor.matmul(
                s_ps[:, i * QW : (i + 1) * QW],
                lhsT=kT[i * E : (i + 1) * E, pr, st * P : (st + 1) * P],
                rhs=qT[i * E : (i + 1) * E, pr, qc * QW : (qc + 1) * QW],
                start=True,
                stop=True,
                tile_position=(i * E, 0),
            )
        s_tiles[(qc, pr, st)] = s_ps

    # ---------------- prelude ----------------
    flat = [(qc, pr, st) for qc in range(QC) for pr in range(2) for st in range(ST)]
    qT_proj(0, 0)
    kT_proj(0, 0)
    emit_scores(*flat[0])
    qT_proj(0, 1)
    v_proj(0)
    v_proj(1)
    o_ps = None
    drain = {}  # (qc, pr) -> (o_f32, rz) awaiting transpose/copy
    emitted = {0}

    def is_dve(j):
        jqc, _, jst = flat[j]
        return jst in DVE_ST_OF_QC[jqc]

    def maybe_scores(j):
        if j < len(flat) and j not in emitted:
            emit_scores(*flat[j])
            emitted.add(j)

    for k, (qc, pr, st) in enumerate(flat):
        maybe_scores(k + 1)
        # When stage k+1 runs its exp on DVE, the scalar engine skips
        # straight from exp(k) to exp(k+2) — emit scores(k+2) ahead of
        # PV(k)/PV(k+1) in PE program order so exp(k+2) is not left waiting
        # behind matmuls that themselves wait on the DVE op.
        # Same treatment at block starts: PV(st0) of the new block waits on
        # the old block's drain copies, and scores(st1) must not queue
        # behind it on the PE.
        if k + 1 < len(flat) and (is_dve(k + 1) or flat[k + 1][2] == 0):
            maybe_scores(k + 2)
        if st == 0:
            o_ps = [
                psum.tile([P, SC, VW], FP32, tag=f"po{i}", bufs=1,
                          name=f"o{i}_{pr}_{qc}")
                for i in range(2)
            ]
        s_ps = s_tiles.pop((qc, pr, st))
        if st in DVE_ST_OF_QC[qc]:
            pi = psb.tile([P, 2 * QW], mybir.dt.int32, tag="pi", bufs=4)
            nc.vector.tensor_scalar(
                out=pi[:], in0=s_ps[:], scalar1=SCH_A, scalar2=SCH_B,
                op0=mybir.AluOpType.mult, op1=mybir.AluOpType.add,
            )
            pv = pi[:].bitcast(BF16)
            p_lhs = [
                [pv[:, 2 * (i * QW + qs * P) + 1 : 2 * (i * QW + (qs + 1) * P) : 2]
                 for qs in range(SC)]
                for i in range(2)
            ]
        else:
            p_sb = psb.tile([P, 2 * QW], BF16, tag="p")
            nc.scalar.activation(p_sb[:], s_ps[:], AF.Exp, scale=float(scale))
            p_lhs = [
                [p_sb[:, i * QW + qs * P : i * QW + (qs + 1) * P]
                 for qs in range(SC)]
                for i in range(2)
            ]
        # transposed PV: O[q, e] with q on partitions; col E accumulates Z[q].
        # start=True zeroes the accumulator's whole PSUM bank, so only the
        # first matmul into each bank (qs==0) may set it; the other q-subtile
        # regions accumulate onto the bank-wide zeros it left behind.
        last_stage = qc == QC - 1 and pr == 1 and st == ST - 1
        order = (
            [(i, qs) for i in range(2) for qs in range(SC)]
            if not last_stage else
            [(i, qs) for qs in range(SC) for i in range(2)]
        )
        for i, qs in order:
            nc.tensor.matmul(
                o_ps[i][:, qs, :],
                lhsT=p_lhs[i][qs],
                rhs=v_sb[:, st, 2 * pr + i, :],
                start=(st == 0 and qs == 0),
                stop=(st == ST - 1 and qs == SC - 1),
                skip_group_check=True,
            )
        # spread remaining phase-A / next-chunk projections under the exp;
        # emitted after PV so they never delay the exp feed
        if qc == 0 and pr == 0:
            if st < 14:
                v_proj(st + 2)
            if st % 4 == 0 and st // 4 < 3:
                kT_proj(st // 4 + 1, 0)
            if st == 2:
                kT_proj(0, 1)
        if qc == 0 and pr == 1 and st in (0, 4, 8):
            kT_proj(st // 4 + 1, 1)
        if pr == 1 and qc + 1 < QC:
            if st == 10:
                qT_proj(qc + 1, 0)
            elif st == 12:
                qT_proj(qc + 1, 1)
        # deferred drain of the PREVIOUS block: PE transposes at st 1..4 (so
        # they never gate this block's scores), oT copy at st 5
        prev = flat[k - ST] if k >= ST else None
        if prev is not None and 1 <= st <= 4:
            ent = drain[(prev[0], prev[1])]
            o_f32, rz = ent[0], ent[1]
            qs = st - 1
            if qs == 0:
                tp = psum.tile([P, SC, P], BF16, tag="pa", bufs=2,
                               name=f"tp_{prev[0]}_{prev[1]}")
                drain[(prev[0], prev[1])] = (o_f32, rz, tp)
            else:
                tp = ent[2]
            o_norm = zp.tile([P, 2, E], BF16, tag="onorm", bufs=4,
                             name=f"on_{prev[1]}_{qs}")
            for i in range(2):
                nc.vector.tensor_scalar_mul(
                    out=o_norm[:, i, :], in0=o_f32[:, qs, i, 0:E],
                    scalar1=rz[:, i, qs : qs + 1, 0],
                )
            nc.tensor.transpose(tp[:, qs, :], o_norm[:], ident[:])
        if prev is not None and st == 5:
            pqc, ppr = prev[0], prev[1]
            tp = drain.pop((pqc, ppr))[2]
            nc.vector.tensor_copy(
                out=oT[:, ppr, pqc * QW : (pqc + 1) * QW],
                in_=tp[:].rearrange("p a b -> p (a b)"),
            )
        if qc > 0 and pr == 0 and st in (6, 9, 11, 14):
            out_proj(qc - 1, {6: 0, 9: 1, 11: 2, 14: 3}[st])
        if st == ST - 1:
            # drain: reciprocal of the Z column, then two copies free the
            # PSUM accumulators fast so the next block's PV is not gated
            rz = zp.tile([P, 2, SC, 1], FP32, tag="rz", bufs=2, name=f"rz_{pr}")
            o_f32 = zp.tile([P, SC, 2, E], FP32, tag="of32", bufs=2,
                            name=f"of_{pr}")
            for i in range(2):
                nc.vector.reciprocal(
                    out=rz[:, i, :, :], in_=o_ps[i][:, :, E : E + 1]
                )
                nc.vector.tensor_copy(
                    out=o_f32[:, :, i, :], in_=o_ps[i][:, :, 0:E]
                )
            drain[(qc, pr)] = (o_f32, rz)
    # ---------------- tail: last block's drain + final out projection ----
    o_f32, rz = drain[(QC - 1, 1)]
    tp = psum.tile([P, SC, P], BF16, tag="pa", bufs=2, name="tp_tail")
    _dma = [nc.sync, nc.scalar, nc.sync, nc.scalar]
    for qs in range(SC):
        o_norm = zp.tile([P, 2, E], BF16, tag="onorm", bufs=4,
                         name=f"on_t_{qs}")
        for i in range(2):
            nc.vector.tensor_scalar_mul(
                out=o_norm[:, i, :], in0=o_f32[:, qs, i, 0:E],
                scalar1=rz[:, i, qs : qs + 1, 0],
            )
        nc.tensor.transpose(tp[:, qs, :], o_norm[:], ident[:])
        nc.vector.tensor_copy(
            out=oT[:, 1, (QC - 1) * QW + qs * P : (QC - 1) * QW + (qs + 1) * P],
            in_=tp[:, qs, :],
        )
        out_proj(QC - 1, qs, dma_eng=_dma[qs], tag=f"po{qs % 2}", bufs=1,
                 cast_eng=nc.scalar)

    for pool in (psum, ocp, zp, psb, xpool, big, wpool, const):
        pool.release()


_NC_CACHE = {}


def _get_nc():
    if "nc" not in _NC_CACHE:
        nc = bacc.Bacc("TRN2", target_bir_lowering=False, debug=False)
        with tile.TileContext(nc) as tc:
            _emit(nc, tc)
        nc.finalize()
        _NC_CACHE["nc"] = nc
    return _NC_CACHE["nc"]


def _shard(inputs):
    import ml_dtypes

    bf16 = lambda a: np.ascontiguousarray(
        np.asarray(a, dtype=np.float32).astype(ml_dtypes.bfloat16)
    )
    f32 = lambda a: np.ascontiguousarray(np.asarray(a), dtype=np.float32)
    # host-side layout prep only (transpose + cast); all FLOPs stay on device
    def tile_x(a):
        # [L, D] -> xT [D, L] -> [SC, P, DC, SW] matching the SBUF tiles
        t = bf16(a).T.reshape(DC, P, SC, SW).transpose(2, 1, 0, 3)
        return np.ascontiguousarray(t)

    xT = {
        name: [tile_x(np.asarray(inputs[key], dtype=np.float32)[b]) for b in range(B)]
        for name, key in (("xqT", "queries"), ("xkT", "keys"), ("xvT", "values"))
    }
    Wq, Wk, Wv, Wo = (
        bf16(inputs["Wq"]),
        bf16(inputs["Wk"]),
        bf16(inputs["Wv"]),
        bf16(inputs["Wo"]),
    )
    def tile_w(w):
        return np.ascontiguousarray(w.reshape(DC, P, EC).transpose(1, 0, 2))

    bq = f32(inputs["bq"])
    in_maps = []
    for c in range(8):
        b, j = c // 2, c % 2
        cs = slice(j * EC, (j + 1) * EC)
        in_maps.append(
            {
                "xqT": xT["xqT"][b],
                "xkT": xT["xkT"][b],
                "xvT": xT["xvT"][b],
                "wq": tile_w(Wq[:, cs]),
                "wk": tile_w(Wk[:, cs]),
                "wv": tile_w(Wv[:, cs]),
                "wo": np.ascontiguousarray(Wo[cs, :].reshape(2, P, D).transpose(1, 0, 2)),
                "bq": np.ascontiguousarray(bq[cs].reshape(2, P).T),
            }
        )
    return in_maps


def _run(inputs, trace=False, **kw):
    nc = _get_nc()
    in_maps = _shard(inputs)
    res = run_bass_kernel_spmd(nc, in_maps, core_ids=list(range(8)), trace=trace, **kw)
    f32 = lambda a: np.asarray(a, dtype=np.float32)
    bv, bo, Wo = f32(inputs["bv"]), f32(inputs["bo"]), f32(inputs["Wo"])
    epilogue = bv @ Wo + bo  # exact: softmax rows sum to 1
    outs = np.stack(
        [
            np.asarray(res.results[2 * b]["out"], dtype=np.float32)
            + np.asarray(res.results[2 * b + 1]["out"], dtype=np.float32)
            + epilogue
            for b in range(B)
        ]
    ).astype(np.float32)
    return outs, res


def kernel(**inputs):
    return _run(inputs)[0]


# revision 57
# speedup vs baseline: 1.0097x; 1.0097x over previous
"""Multi-head attention layer on 8 TRN2 NeuronCores.

Problem: B=4, L=S=2048, D=512, H=8 heads of E=64.
out = softmax(scale * (x_q Wq + bq)(x_k Wk + bk)^T) (x_v Wv + bv) Wo + bo

Sharding: core c = 2*b + j handles batch b, head-half j (4 heads).
Each core computes a partial output projection [2048, 512]; the host sums
the two partials per batch and adds the (bv @ Wo + bo) epilogue.
bk is dropped on-chip (softmax is invariant to a per-row constant shift).

Host prep (layout only, no FLOPs): x inputs are transposed to [D, L] and
cast to bf16 so the kernel needs no on-chip transposes.

Host prep is layout-only (transpose/cast/pre-tiling so every DMA
descriptor covers a full 4KB partition row; the per-queue DMA rate is
descriptor-bound).

Per-core kernel (all matmuls bf16, f32 PSUM accumulation):
  qT    = Wq^T xT + bq  [256e, 2048]  (e on partitions, heads packed 2/ptile)
  kT    = Wk^T xT       [256e, 2048]
  v     = (xT)^T Wv     [2048s, 4, 65] with a trailing ones column per head
  loop qc (q chunks of 512) outer, pr (head pair) inner; per s-tile of 128,
  software-pipelined (scores for stage k+1 are emitted before exp of stage
  k; TWO stages ahead around DVE stages and block starts, where the scalar
  engine would otherwise wait out the 2-slot score-PSUM rotation):
    S^T[s,q]   = kT_h^T @ qT_h       (two row-packed matmuls, tile_position)
    P^T        = exp(scale * S^T)    11 of 16 stages on ScalarE; DVE_ST
                 stages instead use a zero-mean Schraudolph exp on DVE (one
                 tensor_scalar int32(A*s + B); the bf16 high half of the
                 int32 IS exp to ~1.8% rms) so the exp stream runs on two
                 engines concurrently.  The exp is the critical path: per
                 core 16.8M exps at 1/lane/cycle.
    O[q,65]   += P_slice^T @ v_aug_h (transposed PV: 8 matmuls of free size
                 65 instead of 2 of 512 — matmul time is the out free dim —
                 and col 64 accumulates Z per PARTITION q, so softmax
                 normalization becomes a per-partition scalar).  PSUM
                 start=True zeroes the accumulator's whole bank: only the
                 first matmul per bank sets it.
  Drain per (qc, pr): DVE reciprocal of the Z column + 2 copies free the
  PSUM accumulators fast; per-partition tensor_scalar ops normalize into
  o_norm [q, h, e]; PE transposes (deferred into the next block's early
  stages so they never gate its scores) flip to oT [he, q], packed 2 heads
  per 128 partitions.  Output projection per q-tile is then just 2 matmuls
  (contraction 128), emitted inside the next chunk's s-loop; the last
  chunk's runs in a per-q-tile pipelined tail with casts on the (then idle)
  scalar engine.
  out  = oT^T @ Wo -> DRAM (bf16 partials; host sums in f32)
"""

import numpy as np

import concourse.bacc as bacc
import concourse.bass as bass
import concourse.mybir as mybir
import concourse.tile as tile
from concourse.bass_utils import run_bass_kernel_spmd
from concourse.masks import make_identity

B, L, S, D, H = 4, 2048, 2048, 512, 8
E = 64          # head dim
HPC = 4         # heads per core
EC = HPC * E    # 256 model cols per core
P = 128
ST = S // P     # 16 s-tiles
DC = D // P     # 4 d-chunks
QC = 4          # q chunks of 512
QW = 512        # q chunk width
SC = 4          # s chunks of 512 (x dma / projection granularity)
SW = 512
FP32 = mybir.dt.float32
BF16 = mybir.dt.bfloat16
AF = mybir.ActivationFunctionType
VW = E + 1      # v columns per head incl. trailing ones column (gives Z)


def _emit(nc, tc):
    # all inputs pre-tiled on host to the exact SBUF layout so every DMA
    # descriptor covers a full partition row (4KB vs 1KB: the per-queue DMA
    # rate is descriptor-bound)
    xqT = nc.dram_tensor("xqT", [SC, P, DC, SW], BF16, kind="ExternalInput")
    xkT = nc.dram_tensor("xkT", [SC, P, DC, SW], BF16, kind="ExternalInput")
    xvT = nc.dram_tensor("xvT", [SC, P, DC, SW], BF16, kind="ExternalInput")
    wq = nc.dram_tensor("wq", [P, DC, EC], BF16, kind="ExternalInput")
    wk = nc.dram_tensor("wk", [P, DC, EC], BF16, kind="ExternalInput")
    wv = nc.dram_tensor("wv", [P, DC, EC], BF16, kind="ExternalInput")
    wo = nc.dram_tensor("wo", [P, 2, D], BF16, kind="ExternalInput")
    bq = nc.dram_tensor("bq", [P, 2], FP32, kind="ExternalInput")
    out = nc.dram_tensor("out", [L, D], BF16, kind="ExternalOutput")

    const = tc.alloc_tile_pool(name="const", bufs=1)
    wpool = tc.alloc_tile_pool(name="weights", bufs=1)
    big = tc.alloc_tile_pool(name="big", bufs=1)
    xpool = tc.alloc_tile_pool(name="xload", bufs=1)
    psb = tc.alloc_tile_pool(name="pexp", bufs=6)
    zp = tc.alloc_tile_pool(name="znorm", bufs=2)
    ocp = tc.alloc_tile_pool(name="oc", bufs=2)
    psum = tc.alloc_tile_pool(name="psum", bufs=1, space="PSUM")

    # One dma_start per load: a single DMA's descriptors already fan out
    # across all 16 DMA engines, so splitting for bandwidth buys nothing —
    # but every issue costs ~0.6us on the SP sequencer, so loads are merged
    # and ordered needed-first.
    bq_sb = const.tile([P, 2], FP32)
    ident = const.tile([P, P], BF16)

    # weights; layout [128 d_local, dc, EC]
    w_sb = {}
    for name, wt in (("wq", wq), ("wk", wk), ("wv", wv)):
        t = wpool.tile([P, DC, EC], BF16, tag=f"w_{name}", name=f"w_{name}")
        w_sb[name] = t

    def load_w(name, wt, eng):
        eng.dma_start(
            out=w_sb[name][:],
            in_=bass.AP(wt, 0, [[DC * EC, P], [1, DC * EC]]),
        )

    # out projection weights packed 2 heads per 128 partitions: row he of
    # wo_sb[:, pt, :] is Wo row pt*128+he (heads 2pt, 2pt+1 stacked)
    wo_sb = wpool.tile([P, 2, D], BF16, tag="w_wo")

    # x chunk tiles: per (name, sc) a [128, DC, 512] tile
    xch = {"xq": [None] * SC, "xk": [None] * SC, "xv": [None] * SC}

    def load_x(name, dram, sc, eng):
        t = xpool.tile([P, DC, SW], BF16, tag=f"x_{name}_{sc}", name=f"x_{name}_{sc}")
        eng.dma_start(
            out=t[:],
            in_=bass.AP(dram, sc * P * DC * SW, [[DC * SW, P], [1, DC * SW]]),
        )
        xch[name][sc] = t

    # Ramp loads: transfers serialize per HWDGE queue (~2.3us per 512KB),
    # and gpsimd's SWDGE queue is ~3x slower — only tensors needed tens of
    # us in (xk3, wo, xq3) go there.  First-exp critical path: bq/wq/wk +
    # xk0 on SP, xq0 on the scalar queue (free once its engine-state load
    # finishes), both split per d-chunk.
    load_x("xk", xkT, 0, nc.scalar)
    # preload the exp activation-table set during the DMA ramp so the first
    # real exp doesn't pay the ~2.7us ACT_TABLE_LOAD
    warm = const.tile([1, 2], FP32)
    nc.vector.memset(warm[:, 0:1], 0.0)
    nc.scalar.activation(warm[:, 1:2], warm[:, 0:1], AF.Exp)
    load_w("wq", wq, nc.sync)
    load_w("wk", wk, nc.sync)
    load_x("xq", xqT, 0, nc.scalar)
    make_identity(nc, ident[:])
    nc.sync.dma_start(out=bq_sb[:], in_=bass.AP(bq, 0, [[2, P], [1, 2]]))
    load_w("wv", wv, nc.sync)
    load_x("xv", xvT, 0, nc.scalar)
    load_x("xk", xkT, 1, nc.sync)
    load_x("xv", xvT, 1, nc.scalar)
    load_x("xk", xkT, 2, nc.sync)
    load_x("xv", xvT, 2, nc.scalar)
    load_x("xk", xkT, 3, nc.gpsimd)
    load_x("xv", xvT, 3, nc.scalar)
    nc.gpsimd.dma_start(
        out=wo_sb[:], in_=bass.AP(wo, 0, [[2 * D, P], [1, 2 * D]])
    )
    load_x("xq", xqT, 1, nc.sync)
    load_x("xq", xqT, 2, nc.sync)
    load_x("xq", xqT, 3, nc.gpsimd)

    # PE p-state warm-up: throwaway identity matmuls from ~9us until the
    # first projection inputs land (~16.5us), so qT/kT/scores run at ramped
    # clock with no idle gap to reset the p-state
    wu = psum.tile([P, P], FP32, tag="pa", bufs=2, name="warmup")
    for _ in range(70):
        nc.tensor.matmul(wu[:], lhsT=ident[:], rhs=ident[:], start=True, stop=True)

    # persistent activations
    qT = big.tile([P, 2, L], BF16, tag="qT")   # [e_local, ptile, q]
    kT = big.tile([P, 2, S], BF16, tag="kT")
    v_sb = big.tile([P, ST, HPC, VW], BF16, tag="v")  # [s_local, s_tile, h, e+1]
    nc.vector.memset(v_sb[:, :, :, E : E + 1], 1.0)
    oT = big.tile([P, 2, L], BF16, tag="oT")  # [he (2 heads x 64e), pr, q]

    # ---------------- projection emitters ----------------
    def qT_proj(qc, pt):
        ps = psum.tile([P, QW], FP32, tag="pa", bufs=2)
        for dc in range(DC):
            nc.tensor.matmul(
                ps[:],
                lhsT=w_sb["wq"][:, dc, pt * P : (pt + 1) * P],
                rhs=xch["xq"][qc][:, dc, :],
                start=(dc == 0),
                stop=(dc == DC - 1),
            )
        nc.vector.tensor_scalar_add(
            out=qT[:, pt, qc * QW : (qc + 1) * QW],
            in0=ps[:],
            scalar1=bq_sb[:, pt : pt + 1],
        )

    def kT_proj(sc, pt, c0=0, cw=SW):
        ps = psum.tile([P, QW], FP32, tag="pa", bufs=2, name=f"kp_{sc}_{pt}_{c0}")
        for dc in range(DC):
            nc.tensor.matmul(
                ps[:, 0:cw],
                lhsT=w_sb["wk"][:, dc, pt * P : (pt + 1) * P],
                rhs=xch["xk"][sc][:, dc, c0 : c0 + cw],
                start=(dc == 0),
                stop=(dc == DC - 1),
            )
        nc.vector.tensor_copy(
            out=kT[:, pt, sc * SW + c0 : sc * SW + c0 + cw], in_=ps[:, 0:cw]
        )

    def v_proj(st):
        ps = psum.tile([P, EC], FP32, tag="pa", bufs=2)
        for dc in range(DC):
            nc.tensor.matmul(
                ps[:],
                lhsT=xch["xv"][st // 4][:, dc, (st % 4) * P : (st % 4 + 1) * P],
                rhs=w_sb["wv"][:, dc, :],
                start=(dc == 0),
                stop=(dc == DC - 1),
            )
        nc.vector.tensor_copy(
            out=v_sb[:, st, :, 0:E],
            in_=ps[:].rearrange("p (h e) -> p h e", h=HPC),
        )

    def out_proj(qc, qt, dma_eng=None, tag="pa", bufs=2, cast_eng=None):
        ops = psum.tile([P, D], FP32, tag=tag, bufs=bufs, name=f"op_{qc}_{qt}")
        q0 = qc * QW + qt * P
        for pt in range(2):
            nc.tensor.matmul(
                ops[:],
                lhsT=oT[:, pt, q0 : q0 + P],
                rhs=wo_sb[:, pt, :],
                start=(pt == 0),
                stop=(pt == 1),
            )
        o_stage = ocp.tile([P, D], BF16, tag="ostage", bufs=3)
        if cast_eng is None:
            nc.vector.tensor_copy(out=o_stage[:], in_=ops[:])
        else:
            cast_eng.copy(out=o_stage[:], in_=ops[:])
        (dma_eng or nc.sync).dma_start(out=out[q0 : q0 + P, :], in_=o_stage[:])

    # ---------------- attention ----------------
    scale = 1.0 / np.sqrt(E)
    # Schraudolph exp on DVE for DVE_ST s-tiles of each block: exp(scale*s)
    # ~= bf16_high16(int32(A*scale*s + B)); C=482784 zero-means the relative
    # error so the approx s-tiles are not systematically overweighted in the
    # softmax (the ~1.8% rms sawtooth lands on len(DVE_ST)/16 of each row).
    SCH_A = float((1 << 23) / np.log(2.0) * scale)
    SCH_B = float(127 * (1 << 23) + (1 << 15) - 482784)
    # no DVE stages in the PE-bound phase-A chunk (qc==0); 5 per block after
    DVE_ST_OF_QC = {0: (), 1: (2, 5, 8, 11, 14), 2: (2, 5, 8, 11, 14),
                    3: (2, 5, 8, 11, 14)}
    s_tiles = {}

    def emit_scores(qc, pr, st):
        s_ps = psum.tile(
            [P, 2 * QW], FP32, tag="ps", bufs=2, name=f"s_{pr}_{qc}_{st}"
        )
        for i in range(2):
            nc.tensor.matmul(
                s_ps[:, i * QW : (i + 1) * QW],
                lhsT=kT[i * E : (i + 1) * E, pr, st * P : (st + 1) * P],
                rhs=qT[i * E : (i + 1) * E, pr, qc * QW : (qc + 1) * QW],
                start=True,
                stop=True,
                tile_position=(i * E, 0),
            )
        s_tiles[(qc, pr, st)] = s_ps

    # ---------------- prelude ----------------
    flat = [(qc, pr, st) for qc in range(QC) for pr in range(2) for st in range(ST)]
    qT_proj(0, 0)
    kT_proj(0, 0)
    emit_scores(*flat[0])
    qT_proj(0, 1)
    v_proj(0)
    v_proj(1)
    o_ps = None
    drain = {}  # (qc, pr) -> (o_f32, rz) awaiting transpose/copy
    emitted = {0}

    def is_dve(j):
        jqc, _, jst = flat[j]
        return jst in DVE_ST_OF_QC[jqc]

    def maybe_scores(j):
        if j < len(flat) and j not in emitted:
            emit_scores(*flat[j])
            emitted.add(j)

    for k, (qc, pr, st) in enumerate(flat):
        maybe_scores(k + 1)
        # When stage k+1 runs its exp on DVE, the scalar engine skips
        # straight from exp(k) to exp(k+2) — emit scores(k+2) ahead of
        # PV(k)/PV(k+1) in PE program order so exp(k+2) is not left waiting
        # behind matmuls that themselves wait on the DVE op.
        # Same treatment at block starts: PV(st0) of the new block waits on
        # the old block's drain copies, and scores(st1) must not queue
        # behind it on the PE.
        if k + 1 < len(flat) and (is_dve(k + 1) or flat[k + 1][2] == 0):
            maybe_scores(k + 2)
        if st == 0:
            o_ps = [
                psum.tile([P, SC, VW], FP32, tag=f"po{i}", bufs=1,
                          name=f"o{i}_{pr}_{qc}")
                for i in range(2)
            ]
        s_ps = s_tiles.pop((qc, pr, st))
        if st in DVE_ST_OF_QC[qc]:
            pi = psb.tile([P, 2 * QW], mybir.dt.int32, tag="pi", bufs=4)
            nc.vector.tensor_scalar(
                out=pi[:], in0=s_ps[:], scalar1=SCH_A, scalar2=SCH_B,
                op0=mybir.AluOpType.mult, op1=mybir.AluOpType.add,
            )
            pv = pi[:].bitcast(BF16)
            p_lhs = [
                [pv[:, 2 * (i * QW + qs * P) + 1 : 2 * (i * QW + (qs + 1) * P) : 2]
                 for qs in range(SC)]
                for i in range(2)
            ]
        else:
            p_sb = psb.tile([P, 2 * QW], BF16, tag="p")
            nc.scalar.activation(p_sb[:], s_ps[:], AF.Exp, scale=float(scale))
            p_lhs = [
                [p_sb[:, i * QW + qs * P : i * QW + (qs + 1) * P]
                 for qs in range(SC)]
                for i in range(2)
            ]
        # transposed PV: O[q, e] with q on partitions; col E accumulates Z[q].
        # start=True zeroes the accumulator's whole PSUM bank, so only the
        # first matmul into each bank (qs==0) may set it; the other q-subtile
        # regions accumulate onto the bank-wide zeros it left behind.
        last_stage = qc == QC - 1 and pr == 1 and st == ST - 1
        order = (
            [(i, qs) for i in range(2) for qs in range(SC)]
            if not last_stage else
            [(i, qs) for qs in range(SC) for i in range(2)]
        )
        for i, qs in order:
            nc.tensor.matmul(
                o_ps[i][:, qs, :],
                lhsT=p_lhs[i][qs],
                rhs=v_sb[:, st, 2 * pr + i, :],
                start=(st == 0 and qs == 0),
                stop=(st == ST - 1 and qs == SC - 1),
                skip_group_check=True,
            )
        # spread remaining phase-A / next-chunk projections under the exp;
        # emitted after PV so they never delay the exp feed
        if qc == 0 and pr == 0:
            if st < 14:
                v_proj(st + 2)
            if st % 4 == 0 and st // 4 < 3:
                kT_proj(st // 4 + 1, 0)
            if st == 2:
                kT_proj(0, 1)
        if qc == 0 and pr == 1 and st in (0, 4, 8):
            kT_proj(st // 4 + 1, 1)
        if pr == 1 and qc + 1 < QC:
            if st == 10:
                qT_proj(qc + 1, 0)
            elif st == 12:
                qT_proj(qc + 1, 1)
        # deferred drain of the PREVIOUS block: PE transposes at st 1..4 (so
        # they never gate this block's scores), oT copy at st 5
        prev = flat[k - ST] if k >= ST else None
        if prev is not None and 1 <= st <= 4:
            ent = drain[(prev[0], prev[1])]
            o_f32, rz = ent[0], ent[1]
            qs = st - 1
            if qs == 0:
                tp = psum.tile([P, SC, P], BF16, tag="pa", bufs=2,
                               name=f"tp_{prev[0]}_{prev[1]}")
                drain[(prev[0], prev[1])] = (o_f32, rz, tp)
            else:
                tp = ent[2]
            o_norm = zp.tile([P, 2, E], BF16, tag="onorm", bufs=4,
                             name=f"on_{prev[1]}_{qs}")
            for i in range(2):
                nc.vector.tensor_scalar_mul(
                    out=o_norm[:, i, :], in0=o_f32[:, qs, i, 0:E],
                    scalar1=rz[:, i, qs : qs + 1, 0],
                )
            nc.tensor.transpose(tp[:, qs, :], o_norm[:], ident[:])
        if prev is not None and st == 5:
            pqc, ppr = prev[0], prev[1]
            tp = drain.pop((pqc, ppr))[2]
            nc.vector.tensor_copy(
                out=oT[:, ppr, pqc * QW : (pqc + 1) * QW],
                in_=tp[:].rearrange("p a b -> p (a b)"),
            )
        if qc > 0 and pr == 0 and st in (6, 9, 11, 14):
            out_proj(qc - 1, {6: 0, 9: 1, 11: 2, 14: 3}[st])
        if st == ST - 1:
            # drain: reciprocal of the Z column, then two copies free the
            # PSUM accumulators fast so the next block's PV is not gated
            rz = zp.tile([P, 2, SC, 1], FP32, tag="rz", bufs=2, name=f"rz_{pr}")
            o_f32 = zp.tile([P, SC, 2, E], FP32, tag="of32", bufs=2,
                            name=f"of_{pr}")
            for i in range(2):
                nc.vector.reciprocal(
                    out=rz[:, i, :, :], in_=o_ps[i][:, :, E : E + 1]
                )
                nc.vector.tensor_copy(
                    out=o_f32[:, :, i, :], in_=o_ps[i][:, :, 0:E]
                )
            drain[(qc, pr)] = (o_f32, rz)
    # ---------------- tail: last block's drain + final out projection ----
    o_f32, rz = drain[(QC - 1, 1)]
    tp = psum.tile([P, SC, P], BF16, tag="pa", bufs=2, name="tp_tail")
    _dma = [nc.sync, nc.scalar, nc.sync, nc.scalar]
    for qs in range(SC):
        o_norm = zp.tile([P, 2, E], BF16, tag="onorm", bufs=4,
                         name=f"on_t_{qs}")
        for i in range(2):
            nc.vector.tensor_scalar_mul(
                out=o_norm[:, i, :], in0=o_f32[:, qs, i, 0:E],
                scalar1=rz[:, i, qs : qs + 1, 0],
            )
        nc.tensor.transpose(tp[:, qs, :], o_norm[:], ident[:])
        nc.vector.tensor_copy(
            out=oT[:, 1, (QC - 1) * QW + qs * P : (QC - 1) * QW + (qs + 1) * P],
            in_=tp[:, qs, :],
        )
        out_proj(QC - 1, qs, dma_eng=_dma[qs], tag=f"po{qs % 2}", bufs=1,
                 cast_eng=nc.scalar)

    for pool in (psum, ocp, zp, psb, xpool, big, wpool, const):
        pool.release()


_NC_CACHE = {}


def _get_nc():
    if "nc" not in _NC_CACHE:
        nc = bacc.Bacc("TRN2", target_bir_lowering=False, debug=False)
        with tile.TileContext(nc) as tc:
            _emit(nc, tc)
        nc.finalize()
        _NC_CACHE["nc"] = nc
    return _NC_CACHE["nc"]


def _shard(inputs):
    import ml_dtypes

    bf16 = lambda a: np.ascontiguousarray(
        np.asarray(a, dtype=np.float32).astype(ml_dtypes.bfloat16)
    )
    f32 = lambda a: np.ascontiguousarray(np.asarray(a), dtype=np.float32)
    # host-side layout prep only (transpose + cast); all FLOPs stay on device
    def tile_x(a):
        # [L, D] -> xT [D, L] -> [SC, P, DC, SW] matching the SBUF tiles
        t = bf16(a).T.reshape(DC, P, SC, SW).transpose(2, 1, 0, 3)
        return np.ascontiguousarray(t)

    xT = {
        name: [tile_x(np.asarray(inputs[key], dtype=np.float32)[b]) for b in range(B)]
        for name, key in (("xqT", "queries"), ("xkT", "keys"), ("xvT", "values"))
    }
    Wq, Wk, Wv, Wo = (
        bf16(inputs["Wq"]),
        bf16(inputs["Wk"]),
        bf16(inputs["Wv"]),
        bf16(inputs["Wo"]),
    )
    def tile_w(w):
        return np.ascontiguousarray(w.reshape(DC, P, EC).transpose(1, 0, 2))

    bq = f32(inputs["bq"])
    in_maps = []
    for c in range(8):
        b, j = c // 2, c % 2
        cs = slice(j * EC, (j + 1) * EC)
        in_maps.append(
            {
                "xqT": xT["xqT"][b],
                "xkT": xT["xkT"][b],
                "xvT": xT["xvT"][b],
                "wq": tile_w(Wq[:, cs]),
                "wk": tile_w(Wk[:, cs]),
                "wv": tile_w(Wv[:, cs]),
                "wo": np.ascontiguousarray(Wo[cs, :].reshape(2, P, D).transpose(1, 0, 2)),
                "bq": np.ascontiguousarray(bq[cs].reshape(2, P).T),
            }
        )
    return in_maps


def _run(inputs, trace=False, **kw):
    nc = _get_nc()
    in_maps = _shard(inputs)
    res = run_bass_kernel_spmd(nc, in_maps, core_ids=list(range(8)), trace=trace, **kw)
    f32 = lambda a: np.asarray(a, dtype=np.float32)
    bv, bo, Wo = f32(inputs["bv"]), f32(inputs["bo"]), f32(inputs["Wo"])
    epilogue = bv @ Wo + bo  # exact: softmax rows sum to 1
    outs = np.stack(
        [
            np.asarray(res.results[2 * b]["out"], dtype=np.float32)
            + np.asarray(res.results[2 * b + 1]["out"], dtype=np.float32)
            + epilogue
            for b in range(B)
        ]
    ).astype(np.float32)
    return outs, res


def kernel(**inputs):
    return _run(inputs)[0]


# revision 58
# speedup vs baseline: 1.0185x; 1.0087x over previous
"""Multi-head attention layer on 8 TRN2 NeuronCores.

Problem: B=4, L=S=2048, D=512, H=8 heads of E=64.
out = softmax(scale * (x_q Wq + bq)(x_k Wk + bk)^T) (x_v Wv + bv) Wo + bo

Sharding: core c = 2*b + j handles batch b, head-half j (4 heads).
Each core computes a partial output projection [2048, 512]; the host sums
the two partials per batch and adds the (bv @ Wo + bo) epilogue.
bk is dropped on-chip (softmax is invariant to a per-row constant shift).

Host prep (layout only, no FLOPs): x inputs are transposed to [D, L] and
cast to bf16 so the kernel needs no on-chip transposes.

Host prep is layout-only (transpose/cast/pre-tiling so every DMA
descriptor covers a full 4KB partition row; the per-queue DMA rate is
descriptor-bound).

Per-core kernel (all matmuls bf16, f32 PSUM accumulation):
  qT    = Wq^T xT + bq  [256e, 2048]  (e on partitions, heads packed 2/ptile)
  kT    = Wk^T xT       [256e, 2048]
  v     = (xT)^T Wv     [2048s, 4, 65] with a trailing ones column per head
  loop qc (q chunks of 512) outer, pr (head pair) inner; per s-tile of 128,
  software-pipelined (scores for stage k+1 are emitted before exp of stage
  k; TWO stages ahead around DVE stages and block starts, where the scalar
  engine would otherwise wait out the 2-slot score-PSUM rotation):
    S^T[s,q]   = kT_h^T @ qT_h       (two row-packed matmuls, tile_position)
    P^T        = exp(scale * S^T)    11 of 16 stages on ScalarE; DVE_ST
                 stages instead use a zero-mean Schraudolph exp on DVE (one
                 tensor_scalar int32(A*s + B); the bf16 high half of the
                 int32 IS exp to ~1.8% rms) so the exp stream runs on two
                 engines concurrently.  The exp is the critical path: per
                 core 16.8M exps at 1/lane/cycle.
    O[q,65]   += P_slice^T @ v_aug_h (transposed PV: 8 matmuls of free size
                 65 instead of 2 of 512 — matmul time is the out free dim —
                 and col 64 accumulates Z per PARTITION q, so softmax
                 normalization becomes a per-partition scalar).  PSUM
                 start=True zeroes the accumulator's whole bank: only the
                 first matmul per bank sets it.
  Drain per (qc, pr): DVE reciprocal of the Z column + 2 copies free the
  PSUM accumulators fast; per-partition tensor_scalar ops normalize into
  o_norm [q, h, e]; PE transposes (deferred into the next block's early
  stages so they never gate its scores) flip to oT [he, q], packed 2 heads
  per 128 partitions.  Output projection per q-tile is then just 2 matmuls
  (contraction 128), emitted inside the next chunk's s-loop; the last
  chunk's runs in a per-q-tile pipelined tail with casts on the (then idle)
  scalar engine.
  out  = oT^T @ Wo -> DRAM (bf16 partials; host sums in f32)
"""

import numpy as np

import concourse.bacc as bacc
import concourse.bass as bass
import concourse.mybir as mybir
import concourse.tile as tile
from concourse.bass_utils import run_bass_kernel_spmd
from concourse.masks import make_identity

B, L, S, D, H = 4, 2048, 2048, 512, 8
E = 64          # head dim
HPC = 4         # heads per core
EC = HPC * E    # 256 model cols per core
P = 128
ST = S // P     # 16 s-tiles
DC = D // P     # 4 d-chunks
QC = 4          # q chunks of 512
QW = 512        # q chunk width
SC = 4          # s chunks of 512 (x dma / projection granularity)
SW = 512
FP32 = mybir.dt.float32
BF16 = mybir.dt.bfloat16
AF = mybir.ActivationFunctionType
VW = E + 1      # v columns per head incl. trailing ones column (gives Z)


def _emit(nc, tc):
    # all inputs pre-tiled on host to the exact SBUF layout so every DMA
    # descriptor covers a full partition row (4KB vs 1KB: the per-queue DMA
    # rate is descriptor-bound)
    xqT = nc.dram_tensor("xqT", [SC, P, DC, SW], BF16, kind="ExternalInput")
    xkT = nc.dram_tensor("xkT", [SC, P, DC, SW], BF16, kind="ExternalInput")
    xvT = nc.dram_tensor("xvT", [SC, P, DC, SW], BF16, kind="ExternalInput")
    wq = nc.dram_tensor("wq", [P, DC, EC], BF16, kind="ExternalInput")
    wk = nc.dram_tensor("wk", [P, DC, EC], BF16, kind="ExternalInput")
    wv = nc.dram_tensor("wv", [P, DC, EC], BF16, kind="ExternalInput")
    wo = nc.dram_tensor("wo", [P, 2, D], BF16, kind="ExternalInput")
    bq = nc.dram_tensor("bq", [P, 2], FP32, kind="ExternalInput")
    out = nc.dram_tensor("out", [L, D], BF16, kind="ExternalOutput")

    const = tc.alloc_tile_pool(name="const", bufs=1)
    wpool = tc.alloc_tile_pool(name="weights", bufs=1)
    big = tc.alloc_tile_pool(name="big", bufs=1)
    xpool = tc.alloc_tile_pool(name="xload", bufs=1)
    psb = tc.alloc_tile_pool(name="pexp", bufs=6)
    zp = tc.alloc_tile_pool(name="znorm", bufs=2)
    ocp = tc.alloc_tile_pool(name="oc", bufs=2)
    psum = tc.alloc_tile_pool(name="psum", bufs=1, space="PSUM")

    # One dma_start per load: a single DMA's descriptors already fan out
    # across all 16 DMA engines, so splitting for bandwidth buys nothing —
    # but every issue costs ~0.6us on the SP sequencer, so loads are merged
    # and ordered needed-first.
    bq_sb = const.tile([P, 2], FP32)
    ident = const.tile([P, P], BF16)

    # weights; layout [128 d_local, dc, EC]
    w_sb = {}
    for name, wt in (("wq", wq), ("wk", wk), ("wv", wv)):
        t = wpool.tile([P, DC, EC], BF16, tag=f"w_{name}", name=f"w_{name}")
        w_sb[name] = t

    def load_w(name, wt, eng):
        eng.dma_start(
            out=w_sb[name][:],
            in_=bass.AP(wt, 0, [[DC * EC, P], [1, DC * EC]]),
        )

    # out projection weights packed 2 heads per 128 partitions: row he of
    # wo_sb[:, pt, :] is Wo row pt*128+he (heads 2pt, 2pt+1 stacked)
    wo_sb = wpool.tile([P, 2, D], BF16, tag="w_wo")

    # x chunk tiles: per (name, sc) a [128, DC, 512] tile
    xch = {"xq": [None] * SC, "xk": [None] * SC, "xv": [None] * SC}

    def load_x(name, dram, sc, eng):
        t = xpool.tile([P, DC, SW], BF16, tag=f"x_{name}_{sc}", name=f"x_{name}_{sc}")
        eng.dma_start(
            out=t[:],
            in_=bass.AP(dram, sc * P * DC * SW, [[DC * SW, P], [1, DC * SW]]),
        )
        xch[name][sc] = t

    # Ramp loads: transfers serialize per HWDGE queue (~2.3us per 512KB),
    # and gpsimd's SWDGE queue is ~3x slower — only tensors needed tens of
    # us in (xk3, wo, xq3) go there.  First-exp critical path: bq/wq/wk +
    # xk0 on SP, xq0 on the scalar queue (free once its engine-state load
    # finishes), both split per d-chunk.
    load_x("xk", xkT, 0, nc.scalar)
    # preload the exp activation-table set during the DMA ramp so the first
    # real exp doesn't pay the ~2.7us ACT_TABLE_LOAD
    warm = const.tile([1, 2], FP32)
    nc.vector.memset(warm[:, 0:1], 0.0)
    nc.scalar.activation(warm[:, 1:2], warm[:, 0:1], AF.Exp)
    load_w("wq", wq, nc.sync)
    load_w("wk", wk, nc.sync)
    load_x("xq", xqT, 0, nc.scalar)
    make_identity(nc, ident[:])
    nc.sync.dma_start(out=bq_sb[:], in_=bass.AP(bq, 0, [[2, P], [1, 2]]))
    load_w("wv", wv, nc.sync)
    load_x("xv", xvT, 0, nc.scalar)
    load_x("xk", xkT, 1, nc.sync)
    load_x("xv", xvT, 1, nc.scalar)
    load_x("xk", xkT, 2, nc.sync)
    load_x("xv", xvT, 2, nc.scalar)
    load_x("xk", xkT, 3, nc.gpsimd)
    load_x("xv", xvT, 3, nc.scalar)
    nc.gpsimd.dma_start(
        out=wo_sb[:], in_=bass.AP(wo, 0, [[2 * D, P], [1, 2 * D]])
    )
    load_x("xq", xqT, 1, nc.sync)
    load_x("xq", xqT, 2, nc.sync)
    load_x("xq", xqT, 3, nc.gpsimd)

    # PE p-state warm-up: throwaway identity matmuls from ~9us until the
    # first projection inputs land (~16.5us), so qT/kT/scores run at ramped
    # clock with no idle gap to reset the p-state
    wu = psum.tile([P, P], FP32, tag="pa", bufs=2, name="warmup")
    for _ in range(70):
        nc.tensor.matmul(wu[:], lhsT=ident[:], rhs=ident[:], start=True, stop=True)

    # persistent activations
    qT = big.tile([P, 2, L], BF16, tag="qT")   # [e_local, ptile, q]
    kT = big.tile([P, 2, S], BF16, tag="kT")
    v_sb = big.tile([P, ST, HPC, VW], BF16, tag="v")  # [s_local, s_tile, h, e+1]
    nc.vector.memset(v_sb[:, :, :, E : E + 1], 1.0)
    oT = big.tile([P, 2, L], BF16, tag="oT")  # [he (2 heads x 64e), pr, q]

    # ---------------- projection emitters ----------------
    def qT_proj(qc, pt):
        ps = psum.tile([P, QW], FP32, tag="pa", bufs=2)
        for dc in range(DC):
            nc.tensor.matmul(
                ps[:],
                lhsT=w_sb["wq"][:, dc, pt * P : (pt + 1) * P],
                rhs=xch["xq"][qc][:, dc, :],
                start=(dc == 0),
                stop=(dc == DC - 1),
            )
        nc.vector.tensor_scalar_add(
            out=qT[:, pt, qc * QW : (qc + 1) * QW],
            in0=ps[:],
            scalar1=bq_sb[:, pt : pt + 1],
        )

    def kT_proj(sc, pt, c0=0, cw=SW):
        ps = psum.tile([P, QW], FP32, tag="pa", bufs=2, name=f"kp_{sc}_{pt}_{c0}")
        for dc in range(DC):
            nc.tensor.matmul(
                ps[:, 0:cw],
                lhsT=w_sb["wk"][:, dc, pt * P : (pt + 1) * P],
                rhs=xch["xk"][sc][:, dc, c0 : c0 + cw],
                start=(dc == 0),
                stop=(dc == DC - 1),
            )
        nc.vector.tensor_copy(
            out=kT[:, pt, sc * SW + c0 : sc * SW + c0 + cw], in_=ps[:, 0:cw]
        )

    def v_proj(st):
        ps = psum.tile([P, EC], FP32, tag="pa", bufs=2)
        for dc in range(DC):
            nc.tensor.matmul(
                ps[:],
                lhsT=xch["xv"][st // 4][:, dc, (st % 4) * P : (st % 4 + 1) * P],
                rhs=w_sb["wv"][:, dc, :],
                start=(dc == 0),
                stop=(dc == DC - 1),
            )
        nc.vector.tensor_copy(
            out=v_sb[:, st, :, 0:E],
            in_=ps[:].rearrange("p (h e) -> p h e", h=HPC),
        )

    def out_proj(qc, qt, dma_eng=None, tag="pa", bufs=2, cast_eng=None):
        ops = psum.tile([P, D], FP32, tag=tag, bufs=bufs, name=f"op_{qc}_{qt}")
        q0 = qc * QW + qt * P
        for pt in range(2):
            nc.tensor.matmul(
                ops[:],
                lhsT=oT[:, pt, q0 : q0 + P],
                rhs=wo_sb[:, pt, :],
                start=(pt == 0),
                stop=(pt == 1),
            )
        o_stage = ocp.tile([P, D], BF16, tag="ostage", bufs=3)
        if cast_eng is None:
            nc.vector.tensor_copy(out=o_stage[:], in_=ops[:])
        else:
            cast_eng.copy(out=o_stage[:], in_=ops[:])
        (dma_eng or nc.sync).dma_start(out=out[q0 : q0 + P, :], in_=o_stage[:])

    # ---------------- attention ----------------
    scale = 1.0 / np.sqrt(E)
    # Schraudolph exp on DVE for DVE_ST s-tiles of each block: exp(scale*s)
    # ~= bf16_high16(int32(A*scale*s + B)); C=482784 zero-means the relative
    # error so the approx s-tiles are not systematically overweighted in the
    # softmax (the ~1.8% rms sawtooth lands on len(DVE_ST)/16 of each row).
    SCH_A = float((1 << 23) / np.log(2.0) * scale)
    SCH_B = float(127 * (1 << 23) + (1 << 15) - 482784)
    # no DVE stages in the PE-bound phase-A chunk (qc==0); 5 per block after
    DVE_ST_OF_QC = {0: (), 1: (2, 5, 8, 11, 14), 2: (2, 5, 8, 11, 14),
                    3: (2, 5, 8, 11, 14)}
    s_tiles = {}

    def emit_scores(qc, pr, st):
        s_ps = psum.tile(
            [P, 2 * QW], FP32, tag="ps", bufs=2, name=f"s_{pr}_{qc}_{st}"
        )
        for i in range(2):
            nc.tensor.matmul(
                s_ps[:, i * QW : (i + 1) * QW],
                lhsT=kT[i * E : (i + 1) * E, pr, st * P : (st + 1) * P],
                rhs=qT[i * E : (i + 1) * E, pr, qc * QW : (qc + 1) * QW],
                start=True,
                stop=True,
                tile_position=(i * E, 0),
            )
        s_tiles[(qc, pr, st)] = s_ps

    # ---------------- prelude ----------------
    flat = [(qc, pr, st) for qc in range(QC) for pr in range(2) for st in range(ST)]
    qT_proj(0, 0)
    kT_proj(0, 0)
    emit_scores(*flat[0])
    qT_proj(0, 1)
    v_proj(0)
    v_proj(1)
    o_ps = None
    drain = {}  # (qc, pr) -> (o_f32, rz) awaiting transpose/copy
    emitted = {0}

    def is_dve(j):
        jqc, _, jst = flat[j]
        return jst in DVE_ST_OF_QC[jqc]

    def maybe_scores(j):
        if j < len(flat) and j not in emitted:
            emit_scores(*flat[j])
            emitted.add(j)

    for k, (qc, pr, st) in enumerate(flat):
        maybe_scores(k + 1)
        # When stage k+1 runs its exp on DVE, the scalar engine skips
        # straight from exp(k) to exp(k+2) — emit scores(k+2) ahead of
        # PV(k)/PV(k+1) in PE program order so exp(k+2) is not left waiting
        # behind matmuls that themselves wait on the DVE op.
        # Same treatment at block starts: PV(st0) of the new block waits on
        # the old block's drain copies, and scores(st1) must not queue
        # behind it on the PE.
        if k + 1 < len(flat) and (is_dve(k + 1) or flat[k + 1][2] == 0):
            maybe_scores(k + 2)
        if st == 0:
            o_ps = [
                psum.tile([P, SC, VW], FP32, tag=f"po{i}", bufs=1,
                          name=f"o{i}_{pr}_{qc}")
                for i in range(2)
            ]
        s_ps = s_tiles.pop((qc, pr, st))
        if st in DVE_ST_OF_QC[qc]:
            pi = psb.tile([P, 2 * QW], mybir.dt.int32, tag="pi", bufs=4)
            nc.vector.tensor_scalar(
                out=pi[:], in0=s_ps[:], scalar1=SCH_A, scalar2=SCH_B,
                op0=mybir.AluOpType.mult, op1=mybir.AluOpType.add,
            )
            pv = pi[:].bitcast(BF16)
            p_lhs = [
                [pv[:, 2 * (i * QW + qs * P) + 1 : 2 * (i * QW + (qs + 1) * P) : 2]
                 for qs in range(SC)]
                for i in range(2)
            ]
        else:
            p_sb = psb.tile([P, 2 * QW], BF16, tag="p")
            nc.scalar.activation(p_sb[:], s_ps[:], AF.Exp, scale=float(scale))
            p_lhs = [
                [p_sb[:, i * QW + qs * P : i * QW + (qs + 1) * P]
                 for qs in range(SC)]
                for i in range(2)
            ]
        # transposed PV: O[q, e] with q on partitions; col E accumulates Z[q].
        # start=True zeroes the accumulator's whole PSUM bank, so only the
        # first matmul into each bank (qs==0) may set it; the other q-subtile
        # regions accumulate onto the bank-wide zeros it left behind.
        last_stage = qc == QC - 1 and pr == 1 and st == ST - 1
        order = (
            [(i, qs) for i in range(2) for qs in range(SC)]
            if not last_stage else
            [(i, qs) for qs in range(SC) for i in range(2)]
        )
        for i, qs in order:
            nc.tensor.matmul(
                o_ps[i][:, qs, :],
                lhsT=p_lhs[i][qs],
                rhs=v_sb[:, st, 2 * pr + i, :],
                start=(st == 0 and qs == 0),
                stop=(st == ST - 1 and qs == SC - 1),
                skip_group_check=True,
            )
        # spread remaining phase-A / next-chunk projections under the exp;
        # emitted after PV so they never delay the exp feed
        if qc == 0 and pr == 0:
            if st < 14:
                v_proj(st + 2)
            if st % 4 == 0 and st // 4 < 3:
                kT_proj(st // 4 + 1, 0)
            if st == 2:
                kT_proj(0, 1)
        if qc == 0 and pr == 1 and st in (0, 4, 8):
            kT_proj(st // 4 + 1, 1)
        if pr == 1 and qc + 1 < QC:
            if st == 10:
                qT_proj(qc + 1, 0)
            elif st == 12:
                qT_proj(qc + 1, 1)
        # deferred drain of the PREVIOUS block: PE transposes at st 1..4 (so
        # they never gate this block's scores), oT copy at st 5
        prev = flat[k - ST] if k >= ST else None
        if prev is not None and 1 <= st <= 4:
            ent = drain[(prev[0], prev[1])]
            o_f32, rz = ent[0], ent[1]
            qs = st - 1
            if qs == 0:
                tp = psum.tile([P, SC, P], BF16, tag="pa", bufs=2,
                               name=f"tp_{prev[0]}_{prev[1]}")
                drain[(prev[0], prev[1])] = (o_f32, rz, tp)
            else:
                tp = ent[2]
            o_norm = zp.tile([P, 2, E], BF16, tag="onorm", bufs=4,
                             name=f"on_{prev[1]}_{qs}")
            # pr=0 drains have a whole extra block before their oT is read:
            # push their norms/transposes ~2 stages later in scheduler
            # priority so they never sit in front of a DVE Schraudolph op
            # or block the PE behind a not-yet-normalized transpose
            _sp = tc.cur_priority
            if prev[1] == 0:
                tc.cur_priority = _sp + 40
            for i in range(2):
                nc.vector.tensor_scalar_mul(
                    out=o_norm[:, i, :], in0=o_f32[:, qs, i, 0:E],
                    scalar1=rz[:, i, qs : qs + 1, 0],
                )
            nc.tensor.transpose(tp[:, qs, :], o_norm[:], ident[:])
            if prev[1] == 0:
                tc.cur_priority = _sp
        if prev is not None and st == 5:
            pqc, ppr = prev[0], prev[1]
            tp = drain.pop((pqc, ppr))[2]
            nc.vector.tensor_copy(
                out=oT[:, ppr, pqc * QW : (pqc + 1) * QW],
                in_=tp[:].rearrange("p a b -> p (a b)"),
            )
        if qc > 0 and pr == 0 and st in (6, 9, 11, 14):
            out_proj(qc - 1, {6: 0, 9: 1, 11: 2, 14: 3}[st])
        if st == ST - 1:
            # drain: reciprocal of the Z column, then two copies free the
            # PSUM accumulators fast so the next block's PV is not gated
            rz = zp.tile([P, 2, SC, 1], FP32, tag="rz", bufs=2, name=f"rz_{pr}")
            o_f32 = zp.tile([P, SC, 2, E], FP32, tag="of32", bufs=2,
                            name=f"of_{pr}")
            for i in range(2):
                nc.vector.reciprocal(
                    out=rz[:, i, :, :], in_=o_ps[i][:, :, E : E + 1]
                )
                nc.vector.tensor_copy(
                    out=o_f32[:, :, i, :], in_=o_ps[i][:, :, 0:E]
                )
            drain[(qc, pr)] = (o_f32, rz)
    # ---------------- tail: last block's drain + final out projection ----
    o_f32, rz = drain[(QC - 1, 1)]
    tp = psum.tile([P, SC, P], BF16, tag="pa", bufs=2, name="tp_tail")
    _dma = [nc.sync, nc.scalar, nc.sync, nc.scalar]
    for qs in range(SC):
        o_norm = zp.tile([P, 2, E], BF16, tag="onorm", bufs=4,
                         name=f"on_t_{qs}")
        for i in range(2):
            nc.vector.tensor_scalar_mul(
                out=o_norm[:, i, :], in0=o_f32[:, qs, i, 0:E],
                scalar1=rz[:, i, qs : qs + 1, 0],
            )
        nc.tensor.transpose(tp[:, qs, :], o_norm[:], ident[:])
        nc.vector.tensor_copy(
            out=oT[:, 1, (QC - 1) * QW + qs * P : (QC - 1) * QW + (qs + 1) * P],
            in_=tp[:, qs, :],
        )
        out_proj(QC - 1, qs, dma_eng=_dma[qs], tag=f"po{qs % 2}", bufs=1,
                 cast_eng=nc.scalar)

    for pool in (psum, ocp, zp, psb, xpool, big, wpool, const):
        pool.release()


_NC_CACHE = {}


def _get_nc():
    if "nc" not in _NC_CACHE:
        nc = bacc.Bacc("TRN2", target_bir_lowering=False, debug=False)
        with tile.TileContext(nc) as tc:
            _emit(nc, tc)
        nc.finalize()
        _NC_CACHE["nc"] = nc
    return _NC_CACHE["nc"]


def _shard(inputs):
    import ml_dtypes

    bf16 = lambda a: np.ascontiguousarray(
        np.asarray(a, dtype=np.float32).astype(ml_dtypes.bfloat16)
    )
    f32 = lambda a: np.ascontiguousarray(np.asarray(a), dtype=np.float32)
    # host-side layout prep only (transpose + cast); all FLOPs stay on device
    def tile_x(a):
        # [L, D] -> xT [D, L] -> [SC, P, DC, SW] matching the SBUF tiles
        t = bf16(a).T.reshape(DC, P, SC, SW).transpose(2, 1, 0, 3)
        return np.ascontiguousarray(t)

    xT = {
        name: [tile_x(np.asarray(inputs[key], dtype=np.float32)[b]) for b in range(B)]
        for name, key in (("xqT", "queries"), ("xkT", "keys"), ("xvT", "values"))
    }
    Wq, Wk, Wv, Wo = (
        bf16(inputs["Wq"]),
        bf16(inputs["Wk"]),
        bf16(inputs["Wv"]),
        bf16(inputs["Wo"]),
    )
    def tile_w(w):
        return np.ascontiguousarray(w.reshape(DC, P, EC).transpose(1, 0, 2))

    bq = f32(inputs["bq"])
    in_maps = []
    for c in range(8):
        b, j = c // 2, c % 2
        cs = slice(j * EC, (j + 1) * EC)
        in_maps.append(
            {
                "xqT": xT["xqT"][b],
                "xkT": xT["xkT"][b],
                "xvT": xT["xvT"][b],
                "wq": tile_w(Wq[:, cs]),
                "wk": tile_w(Wk[:, cs]),
                "wv": tile_w(Wv[:, cs]),
                "wo": np.ascontiguousarray(Wo[cs, :].reshape(2, P, D).transpose(1, 0, 2)),
                "bq": np.ascontiguousarray(bq[cs].reshape(2, P).T),
            }
        )
    return in_maps


def _run(inputs, trace=False, **kw):
    nc = _get_nc()
    in_maps = _shard(inputs)
    res = run_bass_kernel_spmd(nc, in_maps, core_ids=list(range(8)), trace=trace, **kw)
    f32 = lambda a: np.asarray(a, dtype=np.float32)
    bv, bo, Wo = f32(inputs["bv"]), f32(inputs["bo"]), f32(inputs["Wo"])
    epilogue = bv @ Wo + bo  # exact: softmax rows sum to 1
    outs = np.stack(
        [
            np.asarray(res.results[2 * b]["out"], dtype=np.float32)
            + np.asarray(res.results[2 * b + 1]["out"], dtype=np.float32)
            + epilogue
            for b in range(B)
        ]
    ).astype(np.float32)
    return outs, res


def kernel(**inputs):
    return _run(inputs)[0]


# revision 59
# speedup vs baseline: 1.0248x; 1.0062x over previous
"""Multi-head attention layer on 8 TRN2 NeuronCores.

Problem: B=4, L=S=2048, D=512, H=8 heads of E=64.
out = softmax(scale * (x_q Wq + bq)(x_k Wk + bk)^T) (x_v Wv + bv) Wo + bo

Sharding: core c = 2*b + j handles batch b, head-half j (4 heads).
Each core computes a partial output projection [2048, 512]; the host sums
the two partials per batch and adds the (bv @ Wo + bo) epilogue.
bk is dropped on-chip (softmax is invariant to a per-row constant shift).

Host prep (layout only, no FLOPs): x inputs are transposed to [D, L] and
cast to bf16 so the kernel needs no on-chip transposes.

Host prep is layout-only (transpose/cast/pre-tiling so every DMA
descriptor covers a full 4KB partition row; the per-queue DMA rate is
descriptor-bound).

Per-core kernel (all matmuls bf16, f32 PSUM accumulation):
  qT    = Wq^T xT + bq  [256e, 2048]  (e on partitions, heads packed 2/ptile)
  kT    = Wk^T xT       [256e, 2048]
  v     = (xT)^T Wv     [2048s, 4, 65] with a trailing ones column per head
  loop qc (q chunks of 512) outer, pr (head pair) inner; per s-tile of 128,
  software-pipelined (scores for stage k+1 are emitted before exp of stage
  k; TWO stages ahead around DVE stages and block starts, where the scalar
  engine would otherwise wait out the 2-slot score-PSUM rotation):
    S^T[s,q]   = kT_h^T @ qT_h       (two row-packed matmuls, tile_position)
    P^T        = exp(scale * S^T)    11 of 16 stages on ScalarE; DVE_ST
                 stages instead use a zero-mean Schraudolph exp on DVE (one
                 tensor_scalar int32(A*s + B); the bf16 high half of the
                 int32 IS exp to ~1.8% rms) so the exp stream runs on two
                 engines concurrently.  The exp is the critical path: per
                 core 16.8M exps at 1/lane/cycle.
    O[q,65]   += P_slice^T @ v_aug_h (transposed PV: 8 matmuls of free size
                 65 instead of 2 of 512 — matmul time is the out free dim —
                 and col 64 accumulates Z per PARTITION q, so softmax
                 normalization becomes a per-partition scalar).  PSUM
                 start=True zeroes the accumulator's whole bank: only the
                 first matmul per bank sets it.
  Drain per (qc, pr): DVE reciprocal of the Z column + 2 copies free the
  PSUM accumulators fast; per-partition tensor_scalar ops normalize into
  o_norm [q, h, e]; PE transposes (deferred into the next block's early
  stages so they never gate its scores) flip to oT [he, q], packed 2 heads
  per 128 partitions.  Output projection per q-tile is then just 2 matmuls
  (contraction 128), emitted inside the next chunk's s-loop; the last
  chunk's runs in a per-q-tile pipelined tail with casts on the (then idle)
  scalar engine.
  out  = oT^T @ Wo -> DRAM (bf16 partials; host sums in f32)
"""

import numpy as np

import concourse.bacc as bacc
import concourse.bass as bass
import concourse.mybir as mybir
import concourse.tile as tile
from concourse.bass_utils import run_bass_kernel_spmd
from concourse.masks import make_identity

B, L, S, D, H = 4, 2048, 2048, 512, 8
E = 64          # head dim
HPC = 4         # heads per core
EC = HPC * E    # 256 model cols per core
P = 128
ST = S // P     # 16 s-tiles
DC = D // P     # 4 d-chunks
QC = 4          # q chunks of 512
QW = 512        # q chunk width
SC = 4          # s chunks of 512 (x dma / projection granularity)
SW = 512
FP32 = mybir.dt.float32
BF16 = mybir.dt.bfloat16
AF = mybir.ActivationFunctionType
VW = E + 1      # v columns per head incl. trailing ones column (gives Z)


def _emit(nc, tc):
    # all inputs pre-tiled on host to the exact SBUF layout so every DMA
    # descriptor covers a full partition row (4KB vs 1KB: the per-queue DMA
    # rate is descriptor-bound)
    xqT = nc.dram_tensor("xqT", [SC, P, DC, SW], BF16, kind="ExternalInput")
    xkT = nc.dram_tensor("xkT", [SC, P, DC, SW], BF16, kind="ExternalInput")
    xvT = nc.dram_tensor("xvT", [SC, P, DC, SW], BF16, kind="ExternalInput")
    wq = nc.dram_tensor("wq", [P, DC, EC], BF16, kind="ExternalInput")
    wk = nc.dram_tensor("wk", [P, DC, EC], BF16, kind="ExternalInput")
    wv = nc.dram_tensor("wv", [P, DC, EC], BF16, kind="ExternalInput")
    wo = nc.dram_tensor("wo", [P, 2, D], BF16, kind="ExternalInput")
    bq = nc.dram_tensor("bq", [P, 2], FP32, kind="ExternalInput")
    out = nc.dram_tensor("out", [L, D], BF16, kind="ExternalOutput")

    const = tc.alloc_tile_pool(name="const", bufs=1)
    wpool = tc.alloc_tile_pool(name="weights", bufs=1)
    big = tc.alloc_tile_pool(name="big", bufs=1)
    xpool = tc.alloc_tile_pool(name="xload", bufs=1)
    psb = tc.alloc_tile_pool(name="pexp", bufs=6)
    zp = tc.alloc_tile_pool(name="znorm", bufs=2)
    ocp = tc.alloc_tile_pool(name="oc", bufs=2)
    psum = tc.alloc_tile_pool(name="psum", bufs=1, space="PSUM")

    # One dma_start per load: a single DMA's descriptors already fan out
    # across all 16 DMA engines, so splitting for bandwidth buys nothing —
    # but every issue costs ~0.6us on the SP sequencer, so loads are merged
    # and ordered needed-first.
    bq_sb = const.tile([P, 2], FP32)
    ident = const.tile([P, P], BF16)

    # weights; layout [128 d_local, dc, EC]
    w_sb = {}
    for name, wt in (("wq", wq), ("wk", wk), ("wv", wv)):
        t = wpool.tile([P, DC, EC], BF16, tag=f"w_{name}", name=f"w_{name}")
        w_sb[name] = t

    def load_w(name, wt, eng):
        eng.dma_start(
            out=w_sb[name][:],
            in_=bass.AP(wt, 0, [[DC * EC, P], [1, DC * EC]]),
        )

    # out projection weights packed 2 heads per 128 partitions: row he of
    # wo_sb[:, pt, :] is Wo row pt*128+he (heads 2pt, 2pt+1 stacked)
    wo_sb = wpool.tile([P, 2, D], BF16, tag="w_wo")

    # x chunk tiles: per (name, sc) a [128, DC, 512] tile
    xch = {"xq": [None] * SC, "xk": [None] * SC, "xv": [None] * SC}

    def load_x(name, dram, sc, eng):
        t = xpool.tile([P, DC, SW], BF16, tag=f"x_{name}_{sc}", name=f"x_{name}_{sc}")
        eng.dma_start(
            out=t[:],
            in_=bass.AP(dram, sc * P * DC * SW, [[DC * SW, P], [1, DC * SW]]),
        )
        xch[name][sc] = t

    # Ramp loads: transfers serialize per HWDGE queue (~2.3us per 512KB),
    # and gpsimd's SWDGE queue is ~3x slower — only tensors needed tens of
    # us in (xk3, wo, xq3) go there.  First-exp critical path: bq/wq/wk +
    # xk0 on SP, xq0 on the scalar queue (free once its engine-state load
    # finishes), both split per d-chunk.
    load_x("xk", xkT, 0, nc.scalar)
    # preload the exp activation-table set during the DMA ramp so the first
    # real exp doesn't pay the ~2.7us ACT_TABLE_LOAD
    warm = const.tile([1, 2], FP32)
    nc.vector.memset(warm[:, 0:1], 0.0)
    nc.scalar.activation(warm[:, 1:2], warm[:, 0:1], AF.Exp)
    load_w("wq", wq, nc.sync)
    load_w("wk", wk, nc.sync)
    load_x("xq", xqT, 0, nc.scalar)
    make_identity(nc, ident[:])
    nc.sync.dma_start(out=bq_sb[:], in_=bass.AP(bq, 0, [[2, P], [1, 2]]))
    load_w("wv", wv, nc.sync)
    load_x("xv", xvT, 0, nc.scalar)
    load_x("xk", xkT, 1, nc.sync)
    load_x("xv", xvT, 1, nc.scalar)
    load_x("xk", xkT, 2, nc.sync)
    load_x("xv", xvT, 2, nc.scalar)
    load_x("xk", xkT, 3, nc.gpsimd)
    load_x("xv", xvT, 3, nc.scalar)
    nc.gpsimd.dma_start(
        out=wo_sb[:], in_=bass.AP(wo, 0, [[2 * D, P], [1, 2 * D]])
    )
    load_x("xq", xqT, 1, nc.sync)
    load_x("xq", xqT, 2, nc.sync)
    load_x("xq", xqT, 3, nc.gpsimd)

    # PE p-state warm-up: throwaway identity matmuls from ~9us until the
    # first projection inputs land (~16.5us), so qT/kT/scores run at ramped
    # clock with no idle gap to reset the p-state
    wu = psum.tile([P, P], FP32, tag="pa", bufs=2, name="warmup")
    for _ in range(70):
        nc.tensor.matmul(wu[:], lhsT=ident[:], rhs=ident[:], start=True, stop=True)

    # persistent activations
    qT = big.tile([P, 2, L], BF16, tag="qT")   # [e_local, ptile, q]
    kT = big.tile([P, 2, S], BF16, tag="kT")
    v_sb = big.tile([P, ST, HPC, VW], BF16, tag="v")  # [s_local, s_tile, h, e+1]
    nc.vector.memset(v_sb[:, :, :, E : E + 1], 1.0)
    oT = big.tile([P, 2, L], BF16, tag="oT")  # [he (2 heads x 64e), pr, q]

    # ---------------- projection emitters ----------------
    def qT_proj(qc, pt):
        ps = psum.tile([P, QW], FP32, tag="pa", bufs=2)
        for dc in range(DC):
            nc.tensor.matmul(
                ps[:],
                lhsT=w_sb["wq"][:, dc, pt * P : (pt + 1) * P],
                rhs=xch["xq"][qc][:, dc, :],
                start=(dc == 0),
                stop=(dc == DC - 1),
            )
        nc.vector.tensor_scalar_add(
            out=qT[:, pt, qc * QW : (qc + 1) * QW],
            in0=ps[:],
            scalar1=bq_sb[:, pt : pt + 1],
        )

    def kT_proj(sc, pt, c0=0, cw=SW):
        ps = psum.tile([P, QW], FP32, tag="pa", bufs=2, name=f"kp_{sc}_{pt}_{c0}")
        for dc in range(DC):
            nc.tensor.matmul(
                ps[:, 0:cw],
                lhsT=w_sb["wk"][:, dc, pt * P : (pt + 1) * P],
                rhs=xch["xk"][sc][:, dc, c0 : c0 + cw],
                start=(dc == 0),
                stop=(dc == DC - 1),
            )
        nc.vector.tensor_copy(
            out=kT[:, pt, sc * SW + c0 : sc * SW + c0 + cw], in_=ps[:, 0:cw]
        )

    def v_proj(st):
        ps = psum.tile([P, EC], FP32, tag="pa", bufs=2)
        for dc in range(DC):
            nc.tensor.matmul(
                ps[:],
                lhsT=xch["xv"][st // 4][:, dc, (st % 4) * P : (st % 4 + 1) * P],
                rhs=w_sb["wv"][:, dc, :],
                start=(dc == 0),
                stop=(dc == DC - 1),
            )
        nc.vector.tensor_copy(
            out=v_sb[:, st, :, 0:E],
            in_=ps[:].rearrange("p (h e) -> p h e", h=HPC),
        )

    def out_proj(qc, qt, dma_eng=None, tag="pa", bufs=2, cast_eng=None):
        ops = psum.tile([P, D], FP32, tag=tag, bufs=bufs, name=f"op_{qc}_{qt}")
        q0 = qc * QW + qt * P
        for pt in range(2):
            nc.tensor.matmul(
                ops[:],
                lhsT=oT[:, pt, q0 : q0 + P],
                rhs=wo_sb[:, pt, :],
                start=(pt == 0),
                stop=(pt == 1),
            )
        o_stage = ocp.tile([P, D], BF16, tag="ostage", bufs=3)
        if cast_eng is None:
            nc.vector.tensor_copy(out=o_stage[:], in_=ops[:])
        else:
            cast_eng.copy(out=o_stage[:], in_=ops[:])
        (dma_eng or nc.sync).dma_start(out=out[q0 : q0 + P, :], in_=o_stage[:])

    # ---------------- attention ----------------
    scale = 1.0 / np.sqrt(E)
    # Schraudolph exp on DVE for DVE_ST s-tiles of each block: exp(scale*s)
    # ~= bf16_high16(int32(A*scale*s + B)); C=482784 zero-means the relative
    # error so the approx s-tiles are not systematically overweighted in the
    # softmax (the ~1.8% rms sawtooth lands on len(DVE_ST)/16 of each row).
    SCH_A = float((1 << 23) / np.log(2.0) * scale)
    SCH_B = float(127 * (1 << 23) + (1 << 15) - 482784)
    # no DVE stages in the PE-bound phase-A chunk (qc==0); 5 per block after
    DVE_ST_OF_QC = {0: (), 1: (2, 5, 8, 11, 14), 2: (2, 5, 8, 11, 14),
                    3: (2, 5, 8, 11, 14)}
    s_tiles = {}

    def emit_scores(qc, pr, st):
        s_ps = psum.tile(
            [P, 2 * QW], FP32, tag="ps", bufs=2, name=f"s_{pr}_{qc}_{st}"
        )
        for i in range(2):
            nc.tensor.matmul(
                s_ps[:, i * QW : (i + 1) * QW],
                lhsT=kT[i * E : (i + 1) * E, pr, st * P : (st + 1) * P],
                rhs=qT[i * E : (i + 1) * E, pr, qc * QW : (qc + 1) * QW],
                start=True,
                stop=True,
                tile_position=(i * E, 0),
            )
        s_tiles[(qc, pr, st)] = s_ps

    # ---------------- prelude ----------------
    flat = [(qc, pr, st) for qc in range(QC) for pr in range(2) for st in range(ST)]
    qT_proj(0, 0)
    kT_proj(0, 0)
    emit_scores(*flat[0])
    qT_proj(0, 1)
    v_proj(0)
    v_proj(1)
    o_ps = None
    drain = {}  # (qc, pr) -> (o_f32, rz) awaiting transpose/copy
    emitted = {0}

    def is_dve(j):
        jqc, _, jst = flat[j]
        return jst in DVE_ST_OF_QC[jqc]

    def maybe_scores(j):
        if j < len(flat) and j not in emitted:
            emit_scores(*flat[j])
            emitted.add(j)

    for k, (qc, pr, st) in enumerate(flat):
        maybe_scores(k + 1)
        # When stage k+1 runs its exp on DVE, the scalar engine skips
        # straight from exp(k) to exp(k+2) — emit scores(k+2) ahead of
        # PV(k)/PV(k+1) in PE program order so exp(k+2) is not left waiting
        # behind matmuls that themselves wait on the DVE op.
        # Same treatment at block starts: PV(st0) of the new block waits on
        # the old block's drain copies, and scores(st1) must not queue
        # behind it on the PE.
        if k + 1 < len(flat) and (is_dve(k + 1) or flat[k + 1][2] == 0):
            maybe_scores(k + 2)
        if st == 0:
            o_ps = [
                psum.tile([P, SC, VW], FP32, tag=f"po{i}", bufs=1,
                          name=f"o{i}_{pr}_{qc}")
                for i in range(2)
            ]
        s_ps = s_tiles.pop((qc, pr, st))
        if st in DVE_ST_OF_QC[qc]:
            pi = psb.tile([P, 2 * QW], mybir.dt.int32, tag="pi", bufs=4)
            nc.vector.tensor_scalar(
                out=pi[:], in0=s_ps[:], scalar1=SCH_A, scalar2=SCH_B,
                op0=mybir.AluOpType.mult, op1=mybir.AluOpType.add,
            )
            pv = pi[:].bitcast(BF16)
            p_lhs = [
                [pv[:, 2 * (i * QW + qs * P) + 1 : 2 * (i * QW + (qs + 1) * P) : 2]
                 for qs in range(SC)]
                for i in range(2)
            ]
        else:
            p_sb = psb.tile([P, 2 * QW], BF16, tag="p")
            nc.scalar.activation(p_sb[:], s_ps[:], AF.Exp, scale=float(scale))
            p_lhs = [
                [p_sb[:, i * QW + qs * P : i * QW + (qs + 1) * P]
                 for qs in range(SC)]
                for i in range(2)
            ]
        # transposed PV: O[q, e] with q on partitions; col E accumulates Z[q].
        # start=True zeroes the accumulator's whole PSUM bank, so only the
        # first matmul into each bank (qs==0) may set it; the other q-subtile
        # regions accumulate onto the bank-wide zeros it left behind.
        last_stage = qc == QC - 1 and pr == 1 and st == ST - 1
        order = (
            [(i, qs) for i in range(2) for qs in range(SC)]
            if not last_stage else
            [(i, qs) for qs in range(SC) for i in range(2)]
        )
        for i, qs in order:
            nc.tensor.matmul(
                o_ps[i][:, qs, :],
                lhsT=p_lhs[i][qs],
                rhs=v_sb[:, st, 2 * pr + i, :],
                start=(st == 0 and qs == 0),
                stop=(st == ST - 1 and qs == SC - 1),
                skip_group_check=True,
            )
        # spread remaining phase-A / next-chunk projections under the exp;
        # emitted after PV so they never delay the exp feed
        if qc == 0 and pr == 0:
            if st < 14:
                v_proj(st + 2)
            if st % 4 == 0 and st // 4 < 3:
                kT_proj(st // 4 + 1, 0)
            if st == 2:
                kT_proj(0, 1)
        if qc == 0 and pr == 1 and st in (0, 4, 8):
            kT_proj(st // 4 + 1, 1)
        if pr == 1 and qc + 1 < QC:
            if st == 10:
                qT_proj(qc + 1, 0)
            elif st == 12:
                qT_proj(qc + 1, 1)
        # deferred drain of the PREVIOUS block: PE transposes at st 1..4 (so
        # they never gate this block's scores), oT copy at st 5
        prev = flat[k - ST] if k >= ST else None
        if prev is not None and 1 <= st <= 4:
            ent = drain[(prev[0], prev[1])]
            o_f32, rz = ent[0], ent[1]
            qs = st - 1
            if qs == 0:
                tp = psum.tile([P, SC, P], BF16, tag="pa", bufs=2,
                               name=f"tp_{prev[0]}_{prev[1]}")
                drain[(prev[0], prev[1])] = (o_f32, rz, tp)
            else:
                tp = ent[2]
            o_norm = zp.tile([P, 2, E], BF16, tag="onorm", bufs=4,
                             name=f"on_{prev[1]}_{qs}")
            # pr=0 drains have a whole extra block before their oT is read:
            # push their norms/transposes ~2 stages later in scheduler
            # priority so they never sit in front of a DVE Schraudolph op
            # or block the PE behind a not-yet-normalized transpose
            _sp = tc.cur_priority
            tc.cur_priority = _sp + (80 if prev[1] == 0 else 20)
            for i in range(2):
                nc.vector.tensor_scalar_mul(
                    out=o_norm[:, i, :], in0=o_f32[:, qs, i, 0:E],
                    scalar1=rz[:, i, qs : qs + 1, 0],
                )
            nc.tensor.transpose(tp[:, qs, :], o_norm[:], ident[:])
            tc.cur_priority = _sp
        if prev is not None and st == 5:
            pqc, ppr = prev[0], prev[1]
            tp = drain.pop((pqc, ppr))[2]
            nc.vector.tensor_copy(
                out=oT[:, ppr, pqc * QW : (pqc + 1) * QW],
                in_=tp[:].rearrange("p a b -> p (a b)"),
            )
        if qc > 0 and pr == 0 and st in (6, 9, 11, 14):
            out_proj(qc - 1, {6: 0, 9: 1, 11: 2, 14: 3}[st])
        if st == ST - 1:
            # drain: reciprocal of the Z column, then two copies free the
            # PSUM accumulators fast so the next block's PV is not gated
            rz = zp.tile([P, 2, SC, 1], FP32, tag="rz", bufs=2, name=f"rz_{pr}")
            o_f32 = zp.tile([P, SC, 2, E], FP32, tag="of32", bufs=2,
                            name=f"of_{pr}")
            for i in range(2):
                nc.vector.reciprocal(
                    out=rz[:, i, :, :], in_=o_ps[i][:, :, E : E + 1]
                )
                nc.vector.tensor_copy(
                    out=o_f32[:, :, i, :], in_=o_ps[i][:, :, 0:E]
                )
            drain[(qc, pr)] = (o_f32, rz)
    # ---------------- tail: last block's drain + final out projection ----
    o_f32, rz = drain[(QC - 1, 1)]
    tp = psum.tile([P, SC, P], BF16, tag="pa", bufs=2, name="tp_tail")
    _dma = [nc.sync, nc.scalar, nc.sync, nc.scalar]
    for qs in range(SC):
        o_norm = zp.tile([P, 2, E], BF16, tag="onorm", bufs=4,
                         name=f"on_t_{qs}")
        for i in range(2):
            nc.vector.tensor_scalar_mul(
                out=o_norm[:, i, :], in0=o_f32[:, qs, i, 0:E],
                scalar1=rz[:, i, qs : qs + 1, 0],
            )
        nc.tensor.transpose(tp[:, qs, :], o_norm[:], ident[:])
        nc.vector.tensor_copy(
            out=oT[:, 1, (QC - 1) * QW + qs * P : (QC - 1) * QW + (qs + 1) * P],
            in_=tp[:, qs, :],
        )
        out_proj(QC - 1, qs, dma_eng=_dma[qs], tag=f"po{qs % 2}", bufs=1,
                 cast_eng=nc.scalar)

    for pool in (psum, ocp, zp, psb, xpool, big, wpool, const):
        pool.release()


_NC_CACHE = {}


def _get_nc():
    if "nc" not in _NC_CACHE:
        nc = bacc.Bacc("TRN2", target_bir_lowering=False, debug=False)
        with tile.TileContext(nc) as tc:
            _emit(nc, tc)
        nc.finalize()
        _NC_CACHE["nc"] = nc
    return _NC_CACHE["nc"]


def _shard(inputs):
    import ml_dtypes

    bf16 = lambda a: np.ascontiguousarray(
        np.asarray(a, dtype=np.float32).astype(ml_dtypes.bfloat16)
    )
    f32 = lambda a: np.ascontiguousarray(np.asarray(a), dtype=np.float32)
    # host-side layout prep only (transpose + cast); all FLOPs stay on device
    def tile_x(a):
        # [L, D] -> xT [D, L] -> [SC, P, DC, SW] matching the SBUF tiles
        t = bf16(a).T.reshape(DC, P, SC, SW).transpose(2, 1, 0, 3)
        return np.ascontiguousarray(t)

    xT = {
        name: [tile_x(np.asarray(inputs[key], dtype=np.float32)[b]) for b in range(B)]
        for name, key in (("xqT", "queries"), ("xkT", "keys"), ("xvT", "values"))
    }
    Wq, Wk, Wv, Wo = (
        bf16(inputs["Wq"]),
        bf16(inputs["Wk"]),
        bf16(inputs["Wv"]),
        bf16(inputs["Wo"]),
    )
    def tile_w(w):
        return np.ascontiguousarray(w.reshape(DC, P, EC).transpose(1, 0, 2))

    bq = f32(inputs["bq"])
    in_maps = []
    for c in range(8):
        b, j = c // 2, c % 2
        cs = slice(j * EC, (j + 1) * EC)
        in_maps.append(
            {
                "xqT": xT["xqT"][b],
                "xkT": xT["xkT"][b],
                "xvT": xT["xvT"][b],
                "wq": tile_w(Wq[:, cs]),
                "wk": tile_w(Wk[:, cs]),
                "wv": tile_w(Wv[:, cs]),
                "wo": np.ascontiguousarray(Wo[cs, :].reshape(2, P, D).transpose(1, 0, 2)),
                "bq": np.ascontiguousarray(bq[cs].reshape(2, P).T),
            }
        )
    return in_maps


def _run(inputs, trace=False, **kw):
    nc = _get_nc()
    in_maps = _shard(inputs)
    res = run_bass_kernel_spmd(nc, in_maps, core_ids=list(range(8)), trace=trace, **kw)
    f32 = lambda a: np.asarray(a, dtype=np.float32)
    bv, bo, Wo = f32(inputs["bv"]), f32(inputs["bo"]), f32(inputs["Wo"])
    epilogue = bv @ Wo + bo  # exact: softmax rows sum to 1
    outs = np.stack(
        [
            np.asarray(res.results[2 * b]["out"], dtype=np.float32)
            + np.asarray(res.results[2 * b + 1]["out"], dtype=np.float32)
            + epilogue
            for b in range(B)
        ]
    ).astype(np.float32)
    return outs, res


def kernel(**inputs):
    return _run(inputs)[0]
